# revision 1
# baseline (speedup 1.0000x reference)
"""Trainium2 Bass kernel for streaming dot-product attention with alpha decay.

Math: with e~_s = alpha^{-s} exp(qk_s) the scan becomes a prefix sum computed
as a triangular-ones matmul; QKV_0/Z_0 enter via row-0 fold / K=1 matmul.

Strategy (v12, 85.9us measured on 8 cores; rel err 1.0e-3):
- Host pre-transposes all inputs (qT/kT/ksT/vst; vin chunked with a baked
  ones-column); no device transposes at all.
- fp16 output, DRAM [BL, T+1, N1, D] b-major; output stored as two 512 KB
  DMAs per b issued as soon as each n-half is divided; host casts back.
- R-build: ACT computes ebexp (exp fused with the d-broadcast, dense fp16)
  so DVE's R multiply runs in 2x mode; b=0 uses the plain 1x broadcast
  path instead so its R never waits on the 4.4us ebexp during the ramp.
- All divides run directly on DVE from fp32 PSUM (1x; an ACT-evacuated
  fp16 path measures the same DVE cost here, so it is pure overhead).
- out0 = QKV0/Z0 multiplied on ACT via per-partition scale; GpSimd carries
  only the QKV0 fold and Z0-flatten DMAs (its tensor ops contend with
  DVE's SBUF port and are a net loss).
- ebexp / R-build / QKV0-fold are split into n-halves: halves ACT's
  head-of-line blocking and lets pnum pairs 0-1 start after half 0.

Measured dead ends (do not retry blindly): GpSimd tensor ops (DVE port
contention); [d,n]-major layout (2x divides but worse serialization,
98-100us); single-buffered [T,4,512] psum divides (107us); deeper buffer
pools (ebuf6/rbuf5/obuf5 -> DVE ops slow to 78us busy via SBUF placement
conflicts, 102us).  Next lever: scheduling the [d,n] layout's 2x divides
without the serialization (~8-10us), or trimming the ~10us ramp + ~8us
tail further.  DVE floor here: 32 divides locked at 1x by TRN2's
fp32-only PSUM matmul output.
"""

import math
from contextlib import ExitStack

import numpy as np

import concourse.bass as bass
import concourse.bacc as bacc
import concourse.tile as tile
from concourse import mybir
from concourse.bass_utils import run_bass_kernel_spmd

ALPHA = 0.99
B, N1, N2, D, T = 64, 64, 512, 64, 128
NCORES = 8
BL = B // NCORES
F32 = mybir.dt.float32
F16 = mybir.dt.float16
Exp = mybir.ActivationFunctionType.Exp
Copy = mybir.ActivationFunctionType.Copy


def _build():
    nc = bacc.Bacc("TRN2", target_bir_lowering=False, debug=False)

    qT_d = nc.dram_tensor("qT", [BL, D, N1], F16, kind="ExternalInput")
    kT_d = nc.dram_tensor("kT", [BL, D, N2], F16, kind="ExternalInput")
    vin_d = nc.dram_tensor("vin", [BL, 4, 128, D + 1], F16, kind="ExternalInput")
    ksT_d = nc.dram_tensor("ksT", [BL, D, T], F16, kind="ExternalInput")
    vst_d = nc.dram_tensor("vst", [BL, T, D], F16, kind="ExternalInput")
    tri_d = nc.dram_tensor("tri", [T, T], F16, kind="ExternalInput")
    sb_d = nc.dram_tensor("sbias", [T, 1], F32, kind="ExternalInput")
    out_d = nc.dram_tensor("out", [BL, T + 1, N1, D], F16, kind="ExternalOutput")

    EBEXP_R = set(range(2, BL))  # b0/b1 direct-1x (short ramp); rest ebexp 2x

    with tile.TileContext(nc) as tc, ExitStack() as ctx:
        consts = ctx.enter_context(tc.tile_pool(name="consts", bufs=1))
        inbuf = ctx.enter_context(tc.tile_pool(name="inbuf", bufs=1))
        small = ctx.enter_context(tc.tile_pool(name="small", bufs=8))
        ebuf = ctx.enter_context(tc.tile_pool(name="ebuf", bufs=3))
        rbuf = ctx.enter_context(tc.tile_pool(name="rbuf", bufs=4))
        obuf = ctx.enter_context(tc.tile_pool(name="obuf", bufs=4))
        psum = ctx.enter_context(tc.tile_pool(name="psum", bufs=1, space="PSUM"))

        tri = consts.tile([T, T], F16)
        nc.sync.dma_start(out=tri[:], in_=tri_d[:])
        sbias = consts.tile([T, 1], F32)
        nc.sync.dma_start(out=sbias[:], in_=sb_d[:])

        qT_all = inbuf.tile([D, BL, N1], F16)
        kT_all = inbuf.tile([D, BL, N2], F16)
        ksT_all = inbuf.tile([D, BL, T], F16)
        vin_all = inbuf.tile([128, BL, 4, D + 1], F16)
        vst_all = inbuf.tile([T, BL, D], F16)
        o0all = inbuf.tile([N1, BL, D], F16)

        # b0/b1 input slices land first so compute starts early; rest bulk
        nc.sync.dma_start(out=qT_all[:], in_=qT_d.rearrange("b d n -> d b n"))
        for b in (0, 1):
            e1 = nc.sync if b % 2 == 0 else nc.scalar
            e2 = nc.scalar if b % 2 == 0 else nc.sync
            e1.dma_start(out=kT_all[:, b, :], in_=kT_d[b])
            e2.dma_start(
                out=vin_all[:, b, :, :], in_=vin_d[b].rearrange("c p e -> p c e")
            )
            e1.dma_start(out=ksT_all[:, b, :], in_=ksT_d[b])
            e2.dma_start(out=vst_all[:, b, :], in_=vst_d[b])
        rs = slice(2, BL)
        nc.sync.dma_start(out=kT_all[:, rs, :], in_=kT_d[rs].rearrange("b d m -> d b m"))
        nc.scalar.dma_start(
            out=vin_all[:, rs, :, :], in_=vin_d[rs].rearrange("b c p e -> p b c e")
        )
        nc.sync.dma_start(out=ksT_all[:, rs, :], in_=ksT_d[rs].rearrange("b d t -> d b t"))
        nc.scalar.dma_start(out=vst_all[:, rs, :], in_=vst_d[rs].rearrange("b t d -> t b d"))

        for b in range(BL):
            qT = qT_all[:, b, :]
            use_ebexp = b in EBEXP_R

            # init attention logits: qk[c] [128, 64] = kT_c^T q
            qk_ps = psum.tile([128, 4, N1], F32, tag="pqk", bufs=2)
            for c in range(4):
                nc.tensor.matmul(
                    qk_ps[:, c, :], kT_all[:, b, 128 * c : 128 * (c + 1)], qT,
                    start=True, stop=True,
                )
            qke = small.tile([128, 4, N1], F16, tag="qke")
            nc.scalar.activation(qke[:], qk_ps[:], Exp)

            # [QKV_0 | Z_0]: p0 [64, 65]
            p0 = psum.tile([N1, D + 1], F32, tag="ptr", bufs=2)
            for c in range(4):
                nc.tensor.matmul(
                    p0[:], qke[:, c, :], vin_all[:, b, c, :],
                    start=(c == 0), stop=(c == 3),
                )

            # stream logits ps_s [T, N1]
            ps_s = psum.tile([T, N1], F32, tag="pqk", bufs=2)
            nc.tensor.matmul(ps_s[:], ksT_all[:, b, :], qT, start=True, stop=True)

            # plain eb first: den/reciprocal path never waits on ebexp
            eb = small.tile([T, N1], F16, tag="eb")
            nc.scalar.activation(eb[:], ps_s[:], Exp, bias=sbias[:], scale=1.0)

            # fp16 copy of [QKV0|Z0] on ACT; z0f flatten on gpsimd queue
            p0h = small.tile([N1, D + 1], F16, tag="p0h")
            nc.scalar.activation(p0h[:], p0[:], Copy)
            z0f = small.tile([1, N1], F16, tag="z0f")
            nc.gpsimd.dma_start(out=z0f[:], in_=p0h[:, D : D + 1])

            # out0 = QKV_0/Z_0 into o0all (multiply on ACT via scale)
            rz = small.tile([N1, 1], F32, tag="rz")
            nc.vector.reciprocal(rz[:], p0[:, D : D + 1])
            nc.scalar.activation(o0all[:, b, :], p0[:, 0:D], Copy, scale=rz[:])

            # den + reciprocal (critical path to every divide)
            pden = psum.tile([T, N1], F32, tag="pqk", bufs=2)
            nc.tensor.matmul(pden[:], tri[:], eb[:], start=True, stop=False)
            nc.tensor.matmul(pden[:], tri[0:1, :], z0f[:], start=False, stop=True)
            r_t = small.tile([T, N1], F32, tag="r")
            nc.vector.reciprocal(r_t[:], pden[:])

            # R[s,n,d] = e~[s,n] * v[s,d], in n-halves so pnum pairs 0-1
            # start after half 0 + its QKV0 fold (half 1 builds concurrently)
            R_t = rbuf.tile([T, N1, D], F16, tag="R")
            for hf in range(2):
                hs = slice(32 * hf, 32 * (hf + 1))
                if use_ebexp:
                    ebexp = ebuf.tile([T, 32, D], F16, tag="ebexp")
                    nc.scalar.activation(
                        ebexp[:],
                        ps_s[:, hs, None].broadcast_to([T, 32, D]),
                        Exp, bias=sbias[:], scale=1.0,
                    )
                    nc.vector.tensor_mul(
                        R_t[:, hs, :],
                        ebexp[:],
                        vst_all[:, b, None, :].broadcast_to([T, 32, D]),
                    )
                else:
                    nc.vector.tensor_mul(
                        R_t[:, hs, :],
                        eb[:, hs, None].broadcast_to([T, 32, D]),
                        vst_all[:, b, None, :].broadcast_to([T, 32, D]),
                    )
                nc.gpsimd.dma_start(
                    out=R_t[0:1, hs, :], in_=p0h[hs, None, 0:D],
                    accum_op=mybir.AluOpType.add,
                )

            # numerator matmuls in pairs -> [T, 2, 512] psum; divide per pair
            obig = obuf.tile([T, N1, D], F16, tag="obig")
            for pair in range(4):
                pnum = psum.tile([T, 2, 512], F32, tag="pbig", bufs=2)
                for h in range(2):
                    c = 2 * pair + h
                    nc.tensor.matmul(
                        pnum[:, h, :], tri[:],
                        R_t[:, 8 * c : 8 * (c + 1), :].rearrange(
                            "t n d -> t (n d)"
                        ),
                        start=True, stop=True,
                    )
                ns = slice(16 * pair, 16 * (pair + 1))
                pview = pnum[:].rearrange("t h (n d) -> t (h n) d", d=D)
                nc.vector.tensor_mul(
                    obig[:, ns, :],
                    pview,
                    r_t[:, ns, None].broadcast_to([T, 16, D]),
                )
                if pair % 2 == 1:
                    hs = slice(32 * (pair // 2), 32 * (pair // 2 + 1))
                    eng = nc.sync if b % 2 == 0 else nc.scalar
                    eng.dma_start(
                        out=out_d[b, 1:, hs, :],
                        in_=obig[:, hs, :].rearrange("t n d -> t (n d)"),
                    )

        nc.sync.dma_start(
            out=out_d[:, 0].rearrange("b n d -> n b d"), in_=o0all[:]
        )

    nc.compile()
    return nc


_CACHE = {}


def _get_nc():
    if "nc" not in _CACHE:
        _CACHE["nc"] = _build()
    return _CACHE["nc"]


def _in_maps(q, k_init, v_init, k_stream, v_stream):
    q = np.asarray(q, np.float32).astype(np.float16)
    k_init = np.asarray(k_init, np.float32).astype(np.float16)
    v_init = np.asarray(v_init, np.float32).astype(np.float16)
    k_stream = np.asarray(k_stream, np.float32).astype(np.float16)
    v_stream = np.asarray(v_stream, np.float32).astype(np.float16)

    qT = np.ascontiguousarray(q.transpose(0, 2, 1))            # [B, D, N1]
    kT = np.ascontiguousarray(k_init.transpose(0, 2, 1))       # [B, D, N2]
    vin = np.ones((B, 4, 128, D + 1), np.float16)
    vin[:, :, :, 0:D] = v_init.reshape(B, 4, 128, D)
    ksT = np.ascontiguousarray(k_stream.transpose(1, 2, 0))    # [B, D, T]
    vst = np.ascontiguousarray(v_stream.transpose(1, 0, 2))    # [B, T, D]

    tri = np.triu(np.ones((T, T), np.float32)).astype(np.float16)
    sbias = (np.arange(1, T + 1, dtype=np.float64) * (-math.log(ALPHA))).astype(
        np.float32
    ).reshape(T, 1)
    maps = []
    for i in range(NCORES):
        sl = slice(i * BL, (i + 1) * BL)
        maps.append(
            dict(
                qT=np.ascontiguousarray(qT[sl]),
                kT=np.ascontiguousarray(kT[sl]),
                vin=np.ascontiguousarray(vin[sl]),
                ksT=np.ascontiguousarray(ksT[sl]),
                vst=np.ascontiguousarray(vst[sl]),
                tri=tri,
                sbias=sbias,
            )
        )
    return maps


def run(q, k_init, v_init, attn_mask, k_stream, v_stream, trace=False, **trace_kw):
    """Run on hardware; returns (output, BassKernelResults)."""
    nc = _get_nc()
    maps = _in_maps(q, k_init, v_init, k_stream, v_stream)
    res = run_bass_kernel_spmd(nc, maps, list(range(NCORES)), trace=trace, **trace_kw)
    out = np.concatenate(
        [res.results[i]["out"].transpose(1, 0, 2, 3) for i in range(NCORES)],
        axis=1,
    ).astype(np.float32)
    return out, res


def kernel(q, k_init, v_init, attn_mask, k_stream, v_stream):
    out, _ = run(q, k_init, v_init, attn_mask, k_stream, v_stream, trace=False)
    return out

